# revision 2
# baseline (speedup 1.0000x reference)
"""EnsembleDeepSDF grouped-MLP kernel for 8 Trainium2 NeuronCores.

Strategy:
- Shard data-parallel over points: each type's 16384 points are split 8 ways,
  so every core processes the same (type -> block) schedule => one SPMD program.
- Activations live feature-major (h^T) in SBUF; matmuls run in float32r
  (full-rate, ~tf32 precision); softplus(beta=100) is computed exactly via
  exp/ln on the scalar engine plus one fused custom DVE select-combine:
      H = 100*softplus_beta(y+b) = select(z < -20, 0, z + log1p(exp(-z))),
      z = 100*(y+b)
  The 1/100 and the skip-concat 1/1.414 are folded into the weights host-side.
"""

import os
import sys

import numpy as np

for _p in ("/opt/trn_rl_repo", "/root/.axon_site/_ro/trn_rl_repo"):
    if os.path.isdir(_p) and _p not in sys.path:
        sys.path.insert(0, _p)

import concourse.bacc as bacc
import concourse.mybir as mybir
import concourse.tile as tile
from concourse.bass_utils import run_bass_kernel_spmd

AF = mybir.ActivationFunctionType
dt = mybir.dt

# ---------------------------------------------------------------- custom DVE op
from concourse import dve_ops
from concourse.dve_ops import OPS, DveOp, get_dve_sub_opcode
from concourse.dve_spec import C0, C1, C2, Spec, Src0, Src1, Zero, _has_src1, lower, select
from concourse.dve_uop import DveOpSpec


def _register_sp_combine():
    for op in OPS:
        if op.name == "SP_COMBINE":
            return op
    zz = Src0 * C2 + C0
    spec = Spec(
        body=select(zz < C1, Zero, zz + Src1),
        reference=lambda in0, in1, s0, s1, imm2: np.where(
            in0 * imm2 + s0 < s1, 0.0, in0 * imm2 + s0 + in1
        ),
    )
    op = DveOp("SP_COMBINE", spec, subdim=False, uops_sha={})
    OPS.append(op)
    dve_ops.CUSTOM_DVE_SPECS["SP_COMBINE"] = spec
    dve_ops._SUB_OPCODE_FOR_NAME["SP_COMBINE"] = dve_ops._CUSTOM_DVE_ROW_BASE + len(OPS) - 1
    for ver in ("v3", "v4"):
        compiled = DveOpSpec(
            name="SP_COMBINE",
            opcode=get_dve_sub_opcode("SP_COMBINE"),
            uops=lower(spec, ver=ver),
            rd1_en=_has_src1(spec),
        )
        op.uops_sha[ver] = compiled.sha(ver)
    return op


SP_COMBINE = _register_sp_combine()

# ---------------------------------------------------------------- problem shape
T = 33
D_IN = 35
NL = 8
N_POINTS = T * 16384
NCORES = 8
PC = N_POINTS // NCORES        # 67584 points per core
PTC = 16384 // NCORES          # 2048 points per (type, core)
G = 1024                       # block size (points per pipeline block)
SUBBLK = PTC // G              # 2 blocks per type
NBLK = PC // G                 # 66 blocks per core

DIMS_IN = [35, 200, 200, 200, 200, 200, 200, 200]
DIMS_OUT = [200, 200, 200, 165, 200, 200, 200, 1]
HI_OFF = [0, 200, 400, 600, 765, 965, 1165, 1365]
HI_COLS = 1366
LO_OFF = [None, 0, 200, 400, 565, 765, 965, 1165]
LO_COLS = 1166

TRACE = bool(int(os.environ.get("KERNEL_TRACE", "0")))
LAST_EXEC_NS = None

_CACHE = {}


def _build_nc():
    nc = bacc.Bacc("TRN2", target_bir_lowering=False, debug=False)
    f32 = dt.float32
    f32r = dt.float32r

    xT = nc.dram_tensor("xT", [D_IN, PC], f32, kind="ExternalInput")
    Whi = nc.dram_tensor("Whi", [T, 128, HI_COLS], f32, kind="ExternalInput")
    Wlo = nc.dram_tensor("Wlo", [T, 72, LO_COLS], f32, kind="ExternalInput")
    BnH = nc.dram_tensor("BnH", [T, 128, 8], f32, kind="ExternalInput")
    BnL = nc.dram_tensor("BnL", [T, 72, 8], f32, kind="ExternalInput")
    BpH = nc.dram_tensor("BpH", [T, 128, 8], f32, kind="ExternalInput")
    BpL = nc.dram_tensor("BpL", [T, 72, 8], f32, kind="ExternalInput")
    Y = nc.dram_tensor("Y", [NBLK, G], f32, kind="ExternalOutput")

    NCH = G // 512  # 512-column matmul chunks per block

    with tile.TileContext(nc) as tc:
        with tc.tile_pool(name="w", bufs=2) as wp, \
             tc.tile_pool(name="b", bufs=2) as bp, \
             tc.tile_pool(name="x", bufs=4) as xp, \
             tc.tile_pool(name="h", bufs=3) as hp, \
             tc.tile_pool(name="e", bufs=2) as ep, \
             tc.tile_pool(name="o", bufs=3) as yp, \
             tc.tile_pool(name="ps", bufs=2, space="PSUM") as pp:
            for t in range(T):
                whi = wp.tile([128, HI_COLS], f32r, tag="whi")
                nc.sync.dma_start(whi[:], Whi.ap()[t].bitcast(f32r))
                wlo = wp.tile([72, LO_COLS], f32r, tag="wlo")
                nc.sync.dma_start(wlo[:], Wlo.ap()[t].bitcast(f32r))
                bnh = bp.tile([128, 8], f32, tag="bnh")
                nc.sync.dma_start(bnh[:], BnH.ap()[t])
                bnl = bp.tile([72, 8], f32, tag="bnl")
                nc.sync.dma_start(bnl[:], BnL.ap()[t])
                bph = bp.tile([128, 8], f32, tag="bph")
                nc.sync.dma_start(bph[:], BpH.ap()[t])
                bpl = bp.tile([72, 8], f32, tag="bpl")
                nc.sync.dma_start(bpl[:], BpL.ap()[t])

                for s in range(SUBBLK):
                    bi = t * SUBBLK + s
                    col0 = bi * G
                    xt = xp.tile([D_IN, G], f32r, tag="xt")
                    nc.sync.dma_start(xt[:], xT.ap()[:, col0:col0 + G].bitcast(f32r))

                    prev_hi, prev_hi_rows = xt, D_IN       # K-tile 1 source
                    prev_lo, prev_lo_rows = None, 0        # K-tile 2 source
                    for l in range(NL):
                        O = DIMS_OUT[l]
                        O_hi = min(O, 128)
                        O_lo = O - O_hi
                        ph = pp.tile([128 if l < 7 else 1, G], dt.float32, tag="ph")
                        if O_lo > 0:
                            pl = pp.tile([72, G], dt.float32, tag="pl")
                        else:
                            pl = None

                        otiles = [(0, O_hi, ph)]
                        if O_lo > 0:
                            otiles.append((128, O_lo, pl))
                        for oc0, ocnt, ptile in otiles:
                            for n in range(NCH):
                                c0, c1 = n * 512, (n + 1) * 512
                                srcs = [(whi, HI_OFF[l], prev_hi, prev_hi_rows, 0)]
                                if prev_lo is not None:
                                    srcs.append((wlo, LO_OFF[l], prev_lo, prev_lo_rows, G))
                                nk = len(srcs)
                                for ki, (wt, woff, rt, krows, rcol) in enumerate(srcs):
                                    nc.tensor.matmul(
                                        ptile[0:ocnt, c0:c1],
                                        wt[0:krows, woff + oc0: woff + oc0 + ocnt],
                                        rt[0:krows, rcol + c0: rcol + c1],
                                        start=(ki == 0),
                                        stop=(ki == nk - 1),
                                    )

                        if l < 7:
                            e = ep.tile([128, 2 * G], f32, tag="e")
                            nc.scalar.activation(
                                e[0:128, 0:G], ph[0:128, :], AF.Exp,
                                bias=bnh[:, l:l + 1], scale=-100.0,
                            )
                            if O_lo > 0:
                                nc.scalar.activation(
                                    e[0:O_lo, G:2 * G], pl[0:O_lo, :], AF.Exp,
                                    bias=bnl[0:O_lo, l:l + 1], scale=-100.0,
                                )
                            lt = ep.tile([128, 2 * G], f32, tag="lt")
                            nc.scalar.activation(lt[:, :], e[:, :], AF.Ln, bias=1.0, scale=1.0)

                            ht = hp.tile([128, 2 * G], f32r, tag="H")
                            nc.vector._custom_dve(
                                SP_COMBINE, out=ht[0:128, 0:G], in0=ph[0:128, :],
                                in1=lt[0:128, 0:G], s0=bph[:, l:l + 1], s1=-20.0, imm2=100.0,
                            )
                            if O_lo > 0:
                                nc.vector._custom_dve(
                                    SP_COMBINE, out=ht[0:O_lo, G:2 * G], in0=pl[0:O_lo, :],
                                    in1=lt[0:O_lo, G:2 * G], s0=bpl[0:O_lo, l:l + 1],
                                    s1=-20.0, imm2=100.0,
                                )
                            if l == 3:
                                # skip-concat: x rows become K-rows 165..199 of layer 4
                                nc.sync.dma_start(
                                    ht[37:72, G:2 * G],
                                    xT.ap()[:, col0:col0 + G].bitcast(f32r),
                                )
                            prev_hi, prev_hi_rows = ht, 128
                            prev_lo, prev_lo_rows = ht, 72
                        else:
                            y7 = yp.tile([1, G], f32, tag="y7")
                            nc.vector.tensor_copy(y7[:], ph[0:1, :])
                            nc.sync.dma_start(Y.ap()[bi:bi + 1, :], y7[:])

    nc.compile()
    return nc


def _prep_inputs(x, Ws, bs):
    x = np.ascontiguousarray(np.asarray(x), dtype=np.float32)
    # per-core feature-major x: core c gets, for each type t, points
    # [t*16384 + c*2048, t*16384 + (c+1)*2048)
    xr = x.reshape(T, NCORES, PTC, D_IN)
    xT = np.ascontiguousarray(
        xr.transpose(1, 3, 0, 2).reshape(NCORES, D_IN, PC), dtype=np.float32
    )

    Whi = np.zeros((T, 128, HI_COLS), np.float32)
    Wlo = np.zeros((T, 72, LO_COLS), np.float32)
    for l in range(NL):
        W = np.asarray(Ws[l], dtype=np.float64)
        if l == 0:
            Wl = W
        elif l == 4:
            Wl = W.copy()
            Wl[:, :165, :] /= (100.0 * 1.414)
            Wl[:, 165:, :] /= 1.414
        else:
            Wl = W / 100.0
        di = DIMS_IN[l]
        hi = min(di, 128)
        Whi[:, 0:hi, HI_OFF[l]:HI_OFF[l] + DIMS_OUT[l]] = Wl[:, 0:hi, :]
        if di > 128:
            Wlo[:, 0:di - 128, LO_OFF[l]:LO_OFF[l] + DIMS_OUT[l]] = Wl[:, 128:di, :]

    BnH = np.zeros((T, 128, 8), np.float32)
    BnL = np.zeros((T, 72, 8), np.float32)
    BpH = np.zeros((T, 128, 8), np.float32)
    BpL = np.zeros((T, 72, 8), np.float32)
    for l in range(7):
        b = np.asarray(bs[l], dtype=np.float64)
        O = DIMS_OUT[l]
        O_hi = min(O, 128)
        BnH[:, 0:O_hi, l] = -100.0 * b[:, 0:O_hi]
        BpH[:, 0:O_hi, l] = 100.0 * b[:, 0:O_hi]
        if O > 128:
            BnL[:, 0:O - 128, l] = -100.0 * b[:, 128:O]
            BpL[:, 0:O - 128, l] = 100.0 * b[:, 128:O]
    b7 = np.asarray(bs[7], dtype=np.float32)  # [T, 1]
    return xT, Whi, Wlo, BnH, BnL, BpH, BpL, b7


def kernel(x, type_vec, Ws, bs):
    global LAST_EXEC_NS
    del type_vec  # sorted equal-size groups; segmentation is static

    xT, Whi, Wlo, BnH, BnL, BpH, BpL, b7 = _prep_inputs(x, Ws, bs)

    if "nc" not in _CACHE:
        _CACHE["nc"] = _build_nc()
    nc = _CACHE["nc"]

    in_maps = [
        {
            "xT": xT[c],
            "Whi": Whi, "Wlo": Wlo,
            "BnH": BnH, "BnL": BnL, "BpH": BpH, "BpL": BpL,
        }
        for c in range(NCORES)
    ]
    res = run_bass_kernel_spmd(nc, in_maps, core_ids=list(range(NCORES)), trace=TRACE)
    LAST_EXEC_NS = res.exec_time_ns

    Yall = np.stack([res.results[c]["Y"] for c in range(NCORES)])  # [8, NBLK, G]
    Yr = Yall.reshape(NCORES, T, PTC)
    out = np.ascontiguousarray(Yr.transpose(1, 0, 2).reshape(T, NCORES * PTC))
    out = out + b7  # final-layer bias, broadcast per type
    return out.reshape(N_POINTS, 1).astype(np.float32)
